# revision 5
# baseline (speedup 1.0000x reference)
"""CIN (Compressed Interaction Network) Trainium2 kernel.

Reference computation (per batch row b, emb dim d):
    h0 = x                                  [B, 64, 16]
    h_l[b,n,d] = sum_{i,j} x[b,i,d] * h_{l-1}[b,j,d] * Wl[i*Fi+j, n]
    out = concat([sum_d h1, sum_d h2, sum_d h3], axis=1)   [B, 384]

Strategy (pure data parallel over 8 cores, B_loc = 256):
  * Everything lives in "field-major" layout [field, (b,d)] with
    c = b*16+d as the free/column axis (C = 4096 per core).
  * A CIN layer is z[n, c] = sum_(ij) W[(ij), n] * P[(ij), c] where
    P = Khatri-Rao product P[(i,j), c] = X[i,c]*H[j,c].  P row-chunks
    of 128 are built on VectorE as bf16 tensor_tensor multiplies
    against DMA-broadcast copies of X rows, then contracted on
    TensorE with accumulation in PSUM over the (ij) chunks.
  * Layer 3 only needs the d-summed output, so it is restructured as
    out3[b,:] = vec(G2[b]) @ W2 with G2[b,i,j] = sum_d x[b,i,d]*h2[b,j,d].
    G2 is computed with PE transposes of h2 plus block-diagonal
    matmuls against a host-prepared block-diagonal x tensor, then one
    64-chunk accumulated matmul against W2.  This avoids the whole
    layer-3 Khatri-Rao product (its broadcast + multiply + matmuls).
  * Host side pre-transposes/casts x and pre-chunks the weights
    (host prep is not on the device critical path).
"""

import sys

import numpy as np

try:
    import concourse.bass as bass  # noqa: F401
except ImportError:  # grading env fallback
    sys.path.insert(0, "/opt/trn_rl_repo")

import ml_dtypes
import concourse.bacc as bacc
import concourse.bass as bass
import concourse.mybir as mybir
import concourse.tile as tile
from concourse.bass_utils import run_bass_kernel_spmd

BF16 = mybir.dt.bfloat16
F32 = mybir.dt.float32

B, F0, D = 2048, 64, 16
NCORES = 8
BL = B // NCORES          # 256 batch rows per core
C = BL * D                # 4096 columns (b, d)
FN = 128                  # layer width (all three CIN layers)
CT = 512                  # matmul N tile (one PSUM bank of fp32)
NCT = C // CT             # 8 column tiles
NG = BL // 8              # 32 groups of 8 batch rows (layer-3 path)
L1_CHUNKS = (F0 * F0) // 128   # 32 (two i values per 128-row chunk)
L2_CHUNKS = F0                 # 64 (one i value per 128-row chunk)

_CACHE = {}


def _build_program():
    nc = bacc.Bacc(None, target_bir_lowering=False)

    xt_d = nc.dram_tensor("xt", [F0, C], BF16, kind="ExternalInput")
    x2_d = nc.dram_tensor("x2", [128, C], BF16, kind="ExternalInput")
    xdiag_d = nc.dram_tensor("xdiag", [128, NG * 512], BF16, kind="ExternalInput")
    w0_d = nc.dram_tensor("w0c", [128, L1_CHUNKS * FN], BF16, kind="ExternalInput")
    w1_d = nc.dram_tensor("w1c", [128, L2_CHUNKS * FN], BF16, kind="ExternalInput")
    w2_d = nc.dram_tensor("w2c", [128, L2_CHUNKS * FN], BF16, kind="ExternalInput")
    ident_d = nc.dram_tensor("ident", [128, 128], BF16, kind="ExternalInput")
    out_d = nc.dram_tensor("out_nb", [3, 128, BL], F32, kind="ExternalOutput")

    with tile.TileContext(nc) as tc:
        with (
            tc.tile_pool(name="const", bufs=1) as const,
            tc.tile_pool(name="hbuf", bufs=1) as hbuf,
            tc.tile_pool(name="xb", bufs=4) as xbp,
            tc.tile_pool(name="pkr", bufs=3) as pkr,
            tc.tile_pool(name="outs", bufs=1) as outs,
        ):
            # ---- constant loads -------------------------------------
            x2_sb = const.tile([128, C], BF16)
            nc.sync.dma_start(x2_sb[:], x2_d[:])
            w0_sb = const.tile([128, L1_CHUNKS * FN], BF16)
            nc.sync.dma_start(w0_sb[:], w0_d[:])
            w1_sb = const.tile([128, L2_CHUNKS * FN], BF16)
            nc.sync.dma_start(w1_sb[:], w1_d[:])
            w2_sb = const.tile([128, L2_CHUNKS * FN], BF16)
            nc.sync.dma_start(w2_sb[:], w2_d[:])
            ident_sb = const.tile([128, 128], BF16)
            nc.sync.dma_start(ident_sb[:], ident_d[:])

            h1_sb = hbuf.tile([128, C], BF16, tag="h1")
            h2_sb = hbuf.tile([128, C], BF16, tag="h2")
            out_sb = outs.tile([128, 3 * BL], F32)

            # ---- layers 1 and 2 (Khatri-Rao + accumulated matmul) ----
            with tc.tile_pool(name="zp", bufs=8, space="PSUM") as zp:
                for layer in (0, 1):
                    z_tiles = [
                        zp.tile([128, CT], F32, tag="z", name=f"z_{layer}_{ct}")
                        for ct in range(NCT)
                    ]
                    if layer == 0:
                        nchunk, w_sb, h_in = L1_CHUNKS, w0_sb, x2_sb
                    else:
                        nchunk, w_sb, h_in = L2_CHUNKS, w1_sb, h1_sb

                    for t in range(nchunk):
                        xb = xbp.tile([128, C], BF16, tag="xb")
                        if layer == 0:
                            # rows 0:64 = X[2t] replicated, 64:128 = X[2t+1]
                            nc.sync.dma_start(
                                xb[0:64, :],
                                xt_d[2 * t : 2 * t + 1, :].broadcast_to((64, C)),
                            )
                            nc.sync.dma_start(
                                xb[64:128, :],
                                xt_d[2 * t + 1 : 2 * t + 2, :].broadcast_to((64, C)),
                            )
                        else:
                            nc.sync.dma_start(
                                xb[:], xt_d[t : t + 1, :].broadcast_to((128, C))
                            )

                        p_sb = pkr.tile([128, C], BF16, tag="p")
                        nc.vector.tensor_mul(p_sb[:], h_in[:], xb[:])

                        for ct in range(NCT):
                            nc.tensor.matmul(
                                z_tiles[ct][:],
                                w_sb[:, t * FN : (t + 1) * FN],
                                p_sb[:, ct * CT : (ct + 1) * CT],
                                start=(t == 0),
                                stop=(t == nchunk - 1),
                            )

                    h_out = h1_sb if layer == 0 else h2_sb
                    for ct in range(NCT):
                        nc.scalar.copy(
                            h_out[:, ct * CT : (ct + 1) * CT], z_tiles[ct][:]
                        )
                        nc.vector.reduce_sum(
                            out_sb[:, layer * BL + ct * 32 : layer * BL + (ct + 1) * 32],
                            z_tiles[ct][:].rearrange("p (b d) -> p b d", d=D),
                            axis=mybir.AxisListType.X,
                        )

            # ---- layer 3: out3[n, b] via G2 = sum_d x (x) h2 ---------
            with (
                tc.tile_pool(name="l3sb", bufs=1) as l3sb,
                tc.tile_pool(name="htp", bufs=3, space="PSUM") as htp,
                tc.tile_pool(name="g2p", bufs=3, space="PSUM") as g2p,
                tc.tile_pool(name="o3p", bufs=1, space="PSUM") as o3p,
                tc.tile_pool(name="hts", bufs=3) as hts,
                tc.tile_pool(name="xdg", bufs=4) as xdg,
            ):
                g2t_sb = l3sb.tile([128, NG * 512], BF16)
                for g in range(NG):
                    # transpose h2 block: [128 j, 128 (8b,16d)] -> [(8b,16d), j]
                    ht_ps = htp.tile([128, 128], BF16, tag="ht")
                    nc.tensor.transpose(
                        ht_ps[:], h2_sb[:, g * 128 : (g + 1) * 128], ident_sb[:]
                    )
                    ht_sb = hts.tile([128, 128], BF16, tag="hts")
                    nc.vector.tensor_copy(ht_sb[:], ht_ps[:])

                    xd_sb = xdg.tile([128, 512], BF16, tag="xd")
                    nc.sync.dma_start(xd_sb[:], xdiag_d[:, g * 512 : (g + 1) * 512])

                    # G2T block: out[j, (8b,64i)] = sum_(b',d) h2T x_diag
                    g2_ps = g2p.tile([128, 512], F32, tag="g2")
                    nc.tensor.matmul(g2_ps[:], ht_sb[:], xd_sb[:])
                    nc.vector.tensor_copy(
                        g2t_sb[:, g * 512 : (g + 1) * 512], g2_ps[:]
                    )

                # big contraction: out3T[n, (g,b)] = sum_i W2[i]^T @ G2T_i
                o3_ps = o3p.tile([128, BL], F32)
                g2t_r = g2t_sb[:].rearrange("p (g b i) -> p g b i", b=8, i=F0)
                for i in range(F0):
                    nc.tensor.matmul(
                        o3_ps[:],
                        w2_sb[:, i * FN : (i + 1) * FN],
                        g2t_r[:, :, :, i],
                        start=(i == 0),
                        stop=(i == F0 - 1),
                    )
                nc.scalar.copy(out_sb[:, 2 * BL : 3 * BL], o3_ps[:])

            # ---- store ------------------------------------------------
            for layer in range(3):
                nc.sync.dma_start(
                    out_d[layer], out_sb[:, layer * BL : (layer + 1) * BL]
                )

    nc.finalize()
    return nc


def _prep_inputs(x, W0, W1, W2):
    """Host-side prep: shard x over cores, transpose/cast, chunk weights."""
    bf = ml_dtypes.bfloat16
    xs = np.ascontiguousarray(x).reshape(NCORES, BL, F0, D)

    def chunk_w(W, nchunk):
        # Wc[p, t*FN + n] = W[t*128 + p, n]
        Wc = W.reshape(nchunk, 128, FN).transpose(1, 0, 2).reshape(128, nchunk * FN)
        return np.ascontiguousarray(Wc).astype(bf)

    w0c = chunk_w(W0, L1_CHUNKS)
    w1c = chunk_w(W1, L2_CHUNKS)
    w2c = chunk_w(W2, L2_CHUNKS)
    ident = np.eye(128, dtype=np.float32).astype(bf)

    in_maps = []
    for c in range(NCORES):
        xc = xs[c]                                   # [BL, F0, D]
        xt = xc.transpose(1, 0, 2).reshape(F0, C)    # [i, (b d)]
        xt_bf = xt.astype(bf)
        x2 = np.concatenate([xt_bf, xt_bf], axis=0)  # [128, C]

        # xdiag[(bl', d), (g, bl, i)] = x[g*8+bl, i, d] if bl' == bl else 0
        xd = np.zeros((8, D, NG, 8, F0), dtype=bf)
        xg = xc.reshape(NG, 8, F0, D)                # [g, bl, i, d]
        for bl in range(8):
            xd[bl, :, :, bl, :] = xg[:, bl].transpose(2, 0, 1).astype(bf)
        xdiag = xd.reshape(128, NG * 512)

        in_maps.append(
            {
                "xt": np.ascontiguousarray(xt_bf),
                "x2": np.ascontiguousarray(x2),
                "xdiag": np.ascontiguousarray(xdiag),
                "w0c": w0c,
                "w1c": w1c,
                "w2c": w2c,
                "ident": ident,
            }
        )
    return in_maps


def _postprocess(results):
    # out_nb [3, 128 n, 256 b] per core -> [B, 384]
    outs = [
        np.asarray(r["out_nb"]).transpose(2, 0, 1).reshape(BL, 3 * FN)
        for r in results
    ]
    return np.ascontiguousarray(np.concatenate(outs, axis=0)).astype(np.float32)


def kernel(x, W0, W1, W2, _trace=False, _trace_kwargs=None):
    if "nc" not in _CACHE:
        _CACHE["nc"] = _build_program()
    nc = _CACHE["nc"]
    in_maps = _prep_inputs(
        np.asarray(x, dtype=np.float32),
        np.asarray(W0, dtype=np.float32),
        np.asarray(W1, dtype=np.float32),
        np.asarray(W2, dtype=np.float32),
    )
    kw = {}
    if _trace:
        kw["trace"] = True
        kw.update(_trace_kwargs or {})
    res = run_bass_kernel_spmd(nc, in_maps, core_ids=list(range(NCORES)), **kw)
    out = _postprocess(res.results)
    if _trace:
        _CACHE["last_results"] = res
    return out


# revision 6
# speedup vs baseline: 1.2513x; 1.2513x over previous
"""CIN (Compressed Interaction Network) Trainium2 kernel.

Reference computation (per batch row b, emb dim d):
    h0 = x                                  [B, 64, 16]
    h_l[b,n,d] = sum_{i,j} x[b,i,d] * h_{l-1}[b,j,d] * Wl[i*Fi+j, n]
    out = concat([sum_d h1, sum_d h2, sum_d h3], axis=1)   [B, 384]

Strategy (pure data parallel over 8 cores, B_loc = 256):
  * Everything lives in "field-major" layout [field, (b,d)] with
    c = b*16+d as the free/column axis (C = 4096 per core).
  * A CIN layer is z[n, c] = sum_(ij) W[(ij), n] * P[(ij), c] where
    P = Khatri-Rao product P[(i,j), c] = X[i,c]*H[j,c].  P row-chunks
    of 128 are built on VectorE as bf16 tensor_tensor multiplies
    against DMA-broadcast copies of X rows ("pair tiles": X[2t]
    replicated on partitions 0:64 and X[2t+1] on 64:128), then
    contracted on TensorE with PSUM accumulation over (ij) chunks.
  * Broadcast reuse: columns are processed in two blocks of 2048 and
    the 32 pair tiles of a block are kept resident so layer 1
    (chunk rows (2t, j<64)+(2t+1, j<64)) and layer 2 (half-chunks
    (2t, j-half)+(2t+1, j-half) against partition-duplicated H1)
    consume the same broadcast bytes.  This cuts the DMA broadcast
    traffic from 96 MB to 32 MB per core.
  * Layer 3 only needs the d-summed output, so it is restructured as
    out3[b,:] = vec(G2[b]) @ W2 with G2[b,i,j] = sum_d x[b,i,d]*h2[b,j,d],
    computed with PE transposes of h2 + block-diagonal matmuls against
    a host-prepared block-diagonal x tensor — no layer-3 Khatri-Rao.
  * Host side pre-transposes/casts x and pre-chunks the weights.
"""

import sys

import numpy as np

try:
    import concourse.bass as bass  # noqa: F401
except ImportError:  # grading env fallback
    sys.path.insert(0, "/opt/trn_rl_repo")

import ml_dtypes
import concourse.bacc as bacc
import concourse.bass as bass
import concourse.mybir as mybir
import concourse.tile as tile
from concourse.bass_utils import run_bass_kernel_spmd

BF16 = mybir.dt.bfloat16
F32 = mybir.dt.float32

B, F0, D = 2048, 64, 16
NCORES = 8
BL = B // NCORES          # 256 batch rows per core
C = BL * D                # 4096 columns (b, d)
FN = 128                  # layer width (all three CIN layers)
CT = 512                  # matmul N tile (one PSUM bank of fp32)
CB = 2048                 # column block (broadcast-tile residency)
NBLK = C // CB            # 2
NCT = CB // CT            # 4 column tiles per block
NPAIR = F0 // 2           # 32 broadcast pair tiles
NG = BL // 8              # 32 groups of 8 batch rows (layer-3 path)
L1_CHUNKS = (F0 * F0) // 128   # 32
L2_CHUNKS = F0                 # 64 (pair x j-half)

_CACHE = {}


def _build_program():
    nc = bacc.Bacc(None, target_bir_lowering=False)

    xt_d = nc.dram_tensor("xt", [F0, C], BF16, kind="ExternalInput")
    x2_d = nc.dram_tensor("x2", [128, C], BF16, kind="ExternalInput")
    xdiag_d = nc.dram_tensor("xdiag", [128, NG * 512], BF16, kind="ExternalInput")
    w0_d = nc.dram_tensor("w0c", [128, L1_CHUNKS * FN], BF16, kind="ExternalInput")
    w1_d = nc.dram_tensor("w1c", [128, L2_CHUNKS * FN], BF16, kind="ExternalInput")
    w2_d = nc.dram_tensor("w2c", [128, F0 * FN], BF16, kind="ExternalInput")
    ident_d = nc.dram_tensor("ident", [128, 128], BF16, kind="ExternalInput")
    out_d = nc.dram_tensor("out_nb", [3, 128, BL], F32, kind="ExternalOutput")

    with tile.TileContext(nc) as tc:
        with (
            tc.tile_pool(name="const", bufs=1) as const,
            tc.tile_pool(name="hbuf", bufs=1) as hbuf,
            tc.tile_pool(name="outs", bufs=1) as outs,
        ):
            x2_sb = const.tile([128, C], BF16)
            nc.sync.dma_start(x2_sb[:], x2_d[:])
            w0_sb = const.tile([128, L1_CHUNKS * FN], BF16)
            nc.sync.dma_start(w0_sb[:], w0_d[:])
            w1_sb = const.tile([128, L2_CHUNKS * FN], BF16)
            nc.sync.dma_start(w1_sb[:], w1_d[:])
            ident_sb = const.tile([128, 128], BF16)
            nc.sync.dma_start(ident_sb[:], ident_d[:])

            h1_sb = hbuf.tile([128, C], BF16, tag="h1")
            h2_sb = hbuf.tile([128, C], BF16, tag="h2")
            out_sb = outs.tile([128, 3 * BL], F32)

            # ---- layers 1 and 2, column-blocked with shared pair tiles ----
            with (
                tc.tile_pool(name="pairs", bufs=NPAIR) as pairs,
                tc.tile_pool(name="h2x", bufs=2) as h2xp,
                tc.tile_pool(name="pkr", bufs=3) as pkr,
                tc.tile_pool(name="zp", bufs=8, space="PSUM") as zp,
            ):
                for blk in range(NBLK):
                    c0 = blk * CB
                    pair_tiles = []
                    # ---- layer 1 over this block ----
                    z1 = [
                        zp.tile([128, CT], F32, tag="z", name=f"z1_{blk}_{ct}")
                        for ct in range(NCT)
                    ]
                    for t in range(NPAIR):
                        xb = pairs.tile([128, CB], BF16, tag="xb", name=f"xb_{blk}_{t}")
                        nc.sync.dma_start(
                            xb[0:64, :],
                            xt_d[2 * t : 2 * t + 1, c0 : c0 + CB].broadcast_to(
                                (64, CB)
                            ),
                        )
                        nc.sync.dma_start(
                            xb[64:128, :],
                            xt_d[2 * t + 1 : 2 * t + 2, c0 : c0 + CB].broadcast_to(
                                (64, CB)
                            ),
                        )
                        pair_tiles.append(xb)

                        p_sb = pkr.tile([128, CB], BF16, tag="p", name=f"p1_{blk}_{t}")
                        nc.vector.tensor_mul(p_sb[:], x2_sb[:, c0 : c0 + CB], xb[:])
                        for ct in range(NCT):
                            nc.tensor.matmul(
                                z1[ct][:],
                                w0_sb[:, t * FN : (t + 1) * FN],
                                p_sb[:, ct * CT : (ct + 1) * CT],
                                start=(t == 0),
                                stop=(t == NPAIR - 1),
                            )

                    for ct in range(NCT):
                        cc = c0 + ct * CT
                        nc.scalar.copy(h1_sb[:, cc : cc + CT], z1[ct][:])
                        bo = blk * 128 + ct * 32
                        nc.vector.reduce_sum(
                            out_sb[:, bo : bo + 32],
                            z1[ct][:].rearrange("p (b d) -> p b d", d=D),
                            axis=mybir.AxisListType.X,
                        )

                    # ---- duplicate H1 halves across partitions (via DMA) ----
                    h2x_lo = h2xp.tile([128, CB], BF16, tag="h2xlo", name=f"h2xlo_{blk}")
                    h2x_hi = h2xp.tile([128, CB], BF16, tag="h2xhi", name=f"h2xhi_{blk}")
                    for half, h2x in ((0, h2x_lo), (1, h2x_hi)):
                        src = h1_sb[64 * half : 64 * half + 64, c0 : c0 + CB]
                        nc.sync.dma_start(h2x[0:64, :], src)
                        nc.sync.dma_start(h2x[64:128, :], src)

                    # ---- layer 2 over this block ----
                    z2 = [
                        zp.tile([128, CT], F32, tag="z", name=f"z2_{blk}_{ct}")
                        for ct in range(NCT)
                    ]
                    for t in range(NPAIR):
                        for half, h2x in ((0, h2x_lo), (1, h2x_hi)):
                            k = 2 * t + half
                            p_sb = pkr.tile(
                                [128, CB], BF16, tag="p", name=f"p2_{blk}_{k}"
                            )
                            nc.vector.tensor_mul(p_sb[:], h2x[:], pair_tiles[t][:])
                            for ct in range(NCT):
                                nc.tensor.matmul(
                                    z2[ct][:],
                                    w1_sb[:, k * FN : (k + 1) * FN],
                                    p_sb[:, ct * CT : (ct + 1) * CT],
                                    start=(k == 0),
                                    stop=(k == L2_CHUNKS - 1),
                                )

                    for ct in range(NCT):
                        cc = c0 + ct * CT
                        nc.scalar.copy(h2_sb[:, cc : cc + CT], z2[ct][:])
                        bo = blk * 128 + ct * 32
                        nc.vector.reduce_sum(
                            out_sb[:, BL + bo : BL + bo + 32],
                            z2[ct][:].rearrange("p (b d) -> p b d", d=D),
                            axis=mybir.AxisListType.X,
                        )

            # ---- layer 3: out3[n, b] via G2 = sum_d x (x) h2 ---------
            with (
                tc.tile_pool(name="l3sb", bufs=1) as l3sb,
                tc.tile_pool(name="htp", bufs=3, space="PSUM") as htp,
                tc.tile_pool(name="g2p", bufs=3, space="PSUM") as g2p,
                tc.tile_pool(name="o3p", bufs=1, space="PSUM") as o3p,
                tc.tile_pool(name="hts", bufs=3) as hts,
                tc.tile_pool(name="xdg", bufs=4) as xdg,
            ):
                w2_sb = l3sb.tile([128, F0 * FN], BF16)
                nc.sync.dma_start(w2_sb[:], w2_d[:])
                g2t_sb = l3sb.tile([128, NG * 512], BF16)
                for g in range(NG):
                    # transpose h2 block: [128 j, 128 (8b,16d)] -> [(8b,16d), j]
                    ht_ps = htp.tile([128, 128], BF16, tag="ht", name=f"htps_{g}")
                    nc.tensor.transpose(
                        ht_ps[:], h2_sb[:, g * 128 : (g + 1) * 128], ident_sb[:]
                    )
                    ht_sb = hts.tile([128, 128], BF16, tag="hts", name=f"htsb_{g}")
                    nc.scalar.copy(ht_sb[:], ht_ps[:])

                    xd_sb = xdg.tile([128, 512], BF16, tag="xd", name=f"xd_{g}")
                    nc.sync.dma_start(xd_sb[:], xdiag_d[:, g * 512 : (g + 1) * 512])

                    # G2T block: out[j, (8b,64i)] = sum_(b',d) h2T x_diag
                    g2_ps = g2p.tile([128, 512], F32, tag="g2", name=f"g2ps_{g}")
                    nc.tensor.matmul(g2_ps[:], ht_sb[:], xd_sb[:])
                    nc.scalar.copy(g2t_sb[:, g * 512 : (g + 1) * 512], g2_ps[:])

                # big contraction: out3T[n, (g,b)] = sum_i W2[i]^T @ G2T_i
                o3_ps = o3p.tile([128, BL], F32)
                g2t_r = g2t_sb[:].rearrange("p (g b i) -> p g b i", b=8, i=F0)
                for i in range(F0):
                    nc.tensor.matmul(
                        o3_ps[:],
                        w2_sb[:, i * FN : (i + 1) * FN],
                        g2t_r[:, :, :, i],
                        start=(i == 0),
                        stop=(i == F0 - 1),
                    )
                nc.scalar.copy(out_sb[:, 2 * BL : 3 * BL], o3_ps[:])

            # ---- store ------------------------------------------------
            for layer in range(3):
                nc.sync.dma_start(
                    out_d[layer], out_sb[:, layer * BL : (layer + 1) * BL]
                )

    nc.finalize()
    return nc


def _prep_inputs(x, W0, W1, W2):
    """Host-side prep: shard x over cores, transpose/cast, chunk weights."""
    bf = ml_dtypes.bfloat16
    xs = np.ascontiguousarray(x).reshape(NCORES, BL, F0, D)

    def chunk_w(W, nchunk):
        # Wc[p, t*FN + n] = W[t*128 + p, n]
        Wc = W.reshape(nchunk, 128, FN).transpose(1, 0, 2).reshape(128, nchunk * FN)
        return np.ascontiguousarray(Wc).astype(bf)

    w0c = chunk_w(W0, L1_CHUNKS)
    w2c = chunk_w(W2, F0)
    # W1 chunk (t, half): p<64 -> W1[2t*128 + half*64 + p],
    #                     p>=64 -> W1[(2t+1)*128 + half*64 + (p-64)]
    W1r = W1.reshape(F0, 2, 64, FN)          # [i, half, j_in_half, n]
    w1c = np.zeros((128, L2_CHUNKS * FN), dtype=bf)
    for t in range(NPAIR):
        for half in range(2):
            k = 2 * t + half
            w1c[0:64, k * FN : (k + 1) * FN] = W1r[2 * t, half].astype(bf)
            w1c[64:128, k * FN : (k + 1) * FN] = W1r[2 * t + 1, half].astype(bf)
    ident = np.eye(128, dtype=np.float32).astype(bf)

    in_maps = []
    for c in range(NCORES):
        xc = xs[c]                                   # [BL, F0, D]
        xt = xc.transpose(1, 0, 2).reshape(F0, C)    # [i, (b d)]
        xt_bf = xt.astype(bf)
        x2 = np.concatenate([xt_bf, xt_bf], axis=0)  # [128, C]

        # xdiag[(bl', d), (g, bl, i)] = x[g*8+bl, i, d] if bl' == bl else 0
        xd = np.zeros((8, D, NG, 8, F0), dtype=bf)
        xg = xc.reshape(NG, 8, F0, D)                # [g, bl, i, d]
        for bl in range(8):
            xd[bl, :, :, bl, :] = xg[:, bl].transpose(2, 0, 1).astype(bf)
        xdiag = xd.reshape(128, NG * 512)

        in_maps.append(
            {
                "xt": np.ascontiguousarray(xt_bf),
                "x2": np.ascontiguousarray(x2),
                "xdiag": np.ascontiguousarray(xdiag),
                "w0c": w0c,
                "w1c": np.ascontiguousarray(w1c),
                "w2c": w2c,
                "ident": ident,
            }
        )
    return in_maps


def _postprocess(results):
    # out_nb [3, 128 n, 256 b] per core -> [B, 384]
    outs = [
        np.asarray(r["out_nb"]).transpose(2, 0, 1).reshape(BL, 3 * FN)
        for r in results
    ]
    return np.ascontiguousarray(np.concatenate(outs, axis=0)).astype(np.float32)


def kernel(x, W0, W1, W2, _trace=False, _trace_kwargs=None):
    if "nc" not in _CACHE:
        _CACHE["nc"] = _build_program()
    nc = _CACHE["nc"]
    in_maps = _prep_inputs(
        np.asarray(x, dtype=np.float32),
        np.asarray(W0, dtype=np.float32),
        np.asarray(W1, dtype=np.float32),
        np.asarray(W2, dtype=np.float32),
    )
    kw = {}
    if _trace:
        kw["trace"] = True
        kw.update(_trace_kwargs or {})
    res = run_bass_kernel_spmd(nc, in_maps, core_ids=list(range(NCORES)), **kw)
    out = _postprocess(res.results)
    if _trace:
        _CACHE["last_results"] = res
    return out


# revision 8
# speedup vs baseline: 1.2974x; 1.0369x over previous
"""CIN (Compressed Interaction Network) Trainium2 kernel.

Reference computation (per batch row b, emb dim d):
    h0 = x                                  [B, 64, 16]
    h_l[b,n,d] = sum_{i,j} x[b,i,d] * h_{l-1}[b,j,d] * Wl[i*Fi+j, n]
    out = concat([sum_d h1, sum_d h2, sum_d h3], axis=1)   [B, 384]

Strategy (pure data parallel over 8 cores, B_loc = 256):
  * Everything lives in "field-major" layout [field, (b,d)] with
    c = b*16+d as the free/column axis (C = 4096 per core).
  * A CIN layer is z[n, c] = sum_(ij) W[(ij), n] * P[(ij), c] where
    P = Khatri-Rao product P[(i,j), c] = X[i,c]*H[j,c].  P row-chunks
    of 128 are built on VectorE as bf16 tensor_tensor multiplies
    against "pair tiles" (X[2t] replicated on partitions 0:64 and
    X[2t+1] on 64:128, pre-materialized on the host and DMAed as
    contiguous 512 KB blocks), then contracted on TensorE with PSUM
    accumulation over the (ij) chunks.
  * Pair-tile sharing: layer 1 uses chunk rows (2t, j<64)+(2t+1, j<64)
    and layer 2 uses half-chunks (2t, j-half)+(2t+1, j-half) against
    partition-duplicated H1 halves, so both layers consume the same
    pair-tile bytes (HBM broadcast traffic 64 MB/core vs 96 naive).
  * Layer 3 only needs the d-summed output, so it is restructured as
    out3[b,:] = vec(G2[b]) @ W2 with G2[b,i,j] = sum_d x[b,i,d]*h2[b,j,d],
    computed with PE transposes of h2 + block-diagonal matmuls against
    a host-prepared block-diagonal x tensor — no layer-3 Khatri-Rao.
    Layer-3 work is interleaved per column block to keep TensorE dense.
  * Host side pre-transposes/casts x and pre-chunks the weights.
"""

import sys

import numpy as np

try:
    import concourse.bass as bass  # noqa: F401
except ImportError:  # grading env fallback
    sys.path.insert(0, "/opt/trn_rl_repo")

import ml_dtypes
import concourse.bacc as bacc
import concourse.bass as bass
import concourse.mybir as mybir
import concourse.tile as tile
from concourse.bass_utils import run_bass_kernel_spmd

BF16 = mybir.dt.bfloat16
F32 = mybir.dt.float32

B, F0, D = 2048, 64, 16
NCORES = 8
BL = B // NCORES          # 256 batch rows per core
C = BL * D                # 4096 columns (b, d)
FN = 128                  # layer width (all three CIN layers)
CT = 512                  # matmul N tile (one PSUM bank of fp32)
CB = 2048                 # column block
NBLK = C // CB            # 2
NCT = CB // CT            # 4 column tiles per block
NPAIR = F0 // 2           # 32 pair tiles
NG = BL // 8              # 32 groups of 8 batch rows (layer-3 path)
NGH = NG // NBLK          # 16 layer-3 groups per block
L1_CHUNKS = (F0 * F0) // 128   # 32
L2_CHUNKS = F0                 # 64 (pair x j-half)

_CACHE = {}


def _build_program():
    nc = bacc.Bacc(None, target_bir_lowering=False)

    xtp_d = nc.dram_tensor("xtp", [NBLK, NPAIR, 128, CB], BF16, kind="ExternalInput")
    x2_d = nc.dram_tensor("x2", [128, C], BF16, kind="ExternalInput")
    xdiag_d = nc.dram_tensor("xdiag", [128, NG * 512], BF16, kind="ExternalInput")
    w0_d = nc.dram_tensor("w0c", [128, L1_CHUNKS * FN], BF16, kind="ExternalInput")
    w1_d = nc.dram_tensor("w1c", [128, L2_CHUNKS * FN], BF16, kind="ExternalInput")
    w2_d = nc.dram_tensor("w2c", [128, F0 * FN], BF16, kind="ExternalInput")
    ident_d = nc.dram_tensor("ident", [128, 128], BF16, kind="ExternalInput")
    out_d = nc.dram_tensor("out_nb", [3, 128, BL], F32, kind="ExternalOutput")

    with tile.TileContext(nc) as tc:
        with (
            tc.tile_pool(name="const", bufs=1) as const,
            tc.tile_pool(name="hbuf", bufs=1) as hbuf,
            tc.tile_pool(name="outs", bufs=1) as outs,
            tc.tile_pool(name="xb", bufs=6) as xbp,
            tc.tile_pool(name="h2x", bufs=2) as h2xp,
            tc.tile_pool(name="pkr", bufs=4) as pkr,
            tc.tile_pool(name="zp", bufs=4, space="PSUM") as zp,
            tc.tile_pool(name="l3sb", bufs=1) as l3sb,
            tc.tile_pool(name="l3ps", bufs=3, space="PSUM") as l3ps,
            tc.tile_pool(name="o3p", bufs=1, space="PSUM") as o3p,
            tc.tile_pool(name="hts", bufs=3) as hts,
            tc.tile_pool(name="xdg", bufs=4) as xdg,
        ):
            x2_sb = const.tile([128, C], BF16)
            nc.sync.dma_start(x2_sb[:], x2_d[:])
            w0_sb = const.tile([128, L1_CHUNKS * FN], BF16)
            nc.sync.dma_start(w0_sb[:], w0_d[:])
            w1_sb = const.tile([128, L2_CHUNKS * FN], BF16)
            nc.sync.dma_start(w1_sb[:], w1_d[:])
            w2_sb = const.tile([128, F0 * FN], BF16)
            nc.sync.dma_start(w2_sb[:], w2_d[:])
            ident_sb = const.tile([128, 128], BF16)
            nc.sync.dma_start(ident_sb[:], ident_d[:])

            h1_sb = hbuf.tile([128, C], BF16, tag="h1")
            h2_sb = hbuf.tile([128, C], BF16, tag="h2")
            g2t_sb = l3sb.tile([128, NGH * 512], BF16)
            out_sb = outs.tile([128, 3 * BL], F32)

            for blk in range(NBLK):
                c0 = blk * CB
                # ---------------- layer 1 over this block ----------------
                z1 = [
                    zp.tile([128, CT], F32, tag="z", name=f"z1_{blk}_{ct}")
                    for ct in range(NCT)
                ]
                for t in range(NPAIR):
                    xb = xbp.tile([128, CB], BF16, tag="xb", name=f"xb1_{blk}_{t}")
                    nc.sync.dma_start(xb[:], xtp_d[blk, t])
                    p_sb = pkr.tile([128, CB], BF16, tag="p", name=f"p1_{blk}_{t}")
                    nc.vector.tensor_mul(p_sb[:], x2_sb[:, c0 : c0 + CB], xb[:])
                    for ct in range(NCT):
                        nc.tensor.matmul(
                            z1[ct][:],
                            w0_sb[:, t * FN : (t + 1) * FN],
                            p_sb[:, ct * CT : (ct + 1) * CT],
                            start=(t == 0),
                            stop=(t == NPAIR - 1),
                        )

                for ct in range(NCT):
                    cc = c0 + ct * CT
                    nc.scalar.copy(h1_sb[:, cc : cc + CT], z1[ct][:])
                    bo = blk * 128 + ct * 32
                    nc.vector.reduce_sum(
                        out_sb[:, bo : bo + 32],
                        z1[ct][:].rearrange("p (b d) -> p b d", d=D),
                        axis=mybir.AxisListType.X,
                    )

                # -------- duplicate H1 halves across partitions (DMA) -----
                h2x_lo = h2xp.tile([128, CB], BF16, tag="h2xlo", name=f"h2xlo_{blk}")
                h2x_hi = h2xp.tile([128, CB], BF16, tag="h2xhi", name=f"h2xhi_{blk}")
                for half, h2x in ((0, h2x_lo), (1, h2x_hi)):
                    src = h1_sb[64 * half : 64 * half + 64, c0 : c0 + CB]
                    nc.sync.dma_start(h2x[0:64, :], src)
                    nc.sync.dma_start(h2x[64:128, :], src)

                # ---------------- layer 2 over this block ----------------
                z2 = [
                    zp.tile([128, CT], F32, tag="z", name=f"z2_{blk}_{ct}")
                    for ct in range(NCT)
                ]
                for t in range(NPAIR):
                    xb = xbp.tile([128, CB], BF16, tag="xb", name=f"xb2_{blk}_{t}")
                    nc.sync.dma_start(xb[:], xtp_d[blk, t])
                    for half, h2x in ((0, h2x_lo), (1, h2x_hi)):
                        k = 2 * t + half
                        p_sb = pkr.tile(
                            [128, CB], BF16, tag="p", name=f"p2_{blk}_{k}"
                        )
                        nc.vector.tensor_mul(p_sb[:], h2x[:], xb[:])
                        for ct in range(NCT):
                            nc.tensor.matmul(
                                z2[ct][:],
                                w1_sb[:, k * FN : (k + 1) * FN],
                                p_sb[:, ct * CT : (ct + 1) * CT],
                                start=(k == 0),
                                stop=(k == L2_CHUNKS - 1),
                            )

                for ct in range(NCT):
                    cc = c0 + ct * CT
                    nc.scalar.copy(h2_sb[:, cc : cc + CT], z2[ct][:])
                    bo = blk * 128 + ct * 32
                    nc.vector.reduce_sum(
                        out_sb[:, BL + bo : BL + bo + 32],
                        z2[ct][:].rearrange("p (b d) -> p b d", d=D),
                        axis=mybir.AxisListType.X,
                    )

                # ------- layer 3 for this block's columns (g groups) ------
                for gl in range(NGH):
                    g = blk * NGH + gl
                    # transpose h2 block: [128 j, 128 (8b,16d)] -> [(8b,16d), j]
                    ht_ps = l3ps.tile([128, 128], BF16, tag="l3", name=f"htps_{g}")
                    nc.tensor.transpose(
                        ht_ps[:], h2_sb[:, g * 128 : (g + 1) * 128], ident_sb[:]
                    )
                    ht_sb = hts.tile([128, 128], BF16, tag="hts", name=f"htsb_{g}")
                    nc.scalar.copy(ht_sb[:], ht_ps[:])

                    xd_sb = xdg.tile([128, 512], BF16, tag="xd", name=f"xd_{g}")
                    nc.sync.dma_start(xd_sb[:], xdiag_d[:, g * 512 : (g + 1) * 512])

                    # G2T block: out[j, (8b,64i)] = sum_(b',d) h2T x_diag
                    g2_ps = l3ps.tile([128, 512], F32, tag="l3", name=f"g2ps_{g}")
                    nc.tensor.matmul(g2_ps[:], ht_sb[:], xd_sb[:])
                    nc.scalar.copy(g2t_sb[:, gl * 512 : (gl + 1) * 512], g2_ps[:])

                # big contraction for this block's 128 batch rows:
                # out3T[n, (g, bl)] = sum_i W2[i]^T @ G2T_i
                o3_ps = o3p.tile([128, 128], F32, tag="o3", name=f"o3_{blk}")
                g2t_r = g2t_sb[:].rearrange("p (g b i) -> p g b i", b=8, i=F0)
                for i in range(F0):
                    nc.tensor.matmul(
                        o3_ps[:],
                        w2_sb[:, i * FN : (i + 1) * FN],
                        g2t_r[:, :, :, i],
                        start=(i == 0),
                        stop=(i == F0 - 1),
                    )
                nc.scalar.copy(
                    out_sb[:, 2 * BL + blk * 128 : 2 * BL + (blk + 1) * 128],
                    o3_ps[:],
                )

            # ---- store ------------------------------------------------
            for layer in range(3):
                nc.sync.dma_start(
                    out_d[layer], out_sb[:, layer * BL : (layer + 1) * BL]
                )

    nc.finalize()
    return nc


def _prep_inputs(x, W0, W1, W2):
    """Host-side prep: shard x over cores, transpose/cast, chunk weights."""
    bf = ml_dtypes.bfloat16
    xs = np.ascontiguousarray(x).reshape(NCORES, BL, F0, D)

    def chunk_w(W, nchunk):
        # Wc[p, t*FN + n] = W[t*128 + p, n]
        Wc = W.reshape(nchunk, 128, FN).transpose(1, 0, 2).reshape(128, nchunk * FN)
        return np.ascontiguousarray(Wc).astype(bf)

    w0c = chunk_w(W0, L1_CHUNKS)
    w2c = chunk_w(W2, F0)
    # W1 chunk (t, half): p<64 -> W1[2t*128 + half*64 + p],
    #                     p>=64 -> W1[(2t+1)*128 + half*64 + (p-64)]
    W1r = W1.reshape(F0, 2, 64, FN)          # [i, half, j_in_half, n]
    w1c = np.zeros((128, L2_CHUNKS * FN), dtype=bf)
    for t in range(NPAIR):
        for half in range(2):
            k = 2 * t + half
            w1c[0:64, k * FN : (k + 1) * FN] = W1r[2 * t, half].astype(bf)
            w1c[64:128, k * FN : (k + 1) * FN] = W1r[2 * t + 1, half].astype(bf)
    ident = np.eye(128, dtype=np.float32).astype(bf)

    in_maps = []
    for c in range(NCORES):
        xc = xs[c]                                   # [BL, F0, D]
        xt = xc.transpose(1, 0, 2).reshape(F0, C)    # [i, (b d)]
        xt_bf = xt.astype(bf)
        x2 = np.concatenate([xt_bf, xt_bf], axis=0)  # [128, C]

        # pre-replicated pair tiles: xtp[blk, t] = [64 x X[2t]; 64 x X[2t+1]]
        xtb = xt_bf.reshape(F0, NBLK, CB)            # [i, blk, cb]
        xtp = np.repeat(xtb[:, None, :, :], 64, axis=1)   # [i, 64, blk, cb]
        xtp = (
            xtp.reshape(NPAIR, 128, NBLK, CB)
            .transpose(2, 0, 1, 3)                   # [blk, t, 128, cb]
            .copy()
        )

        # xdiag[(bl', d), (g, bl, i)] = x[g*8+bl, i, d] if bl' == bl else 0
        xd = np.zeros((8, D, NG, 8, F0), dtype=bf)
        xg = xc.reshape(NG, 8, F0, D)                # [g, bl, i, d]
        for bl in range(8):
            xd[bl, :, :, bl, :] = xg[:, bl].transpose(2, 0, 1).astype(bf)
        xdiag = xd.reshape(128, NG * 512)

        in_maps.append(
            {
                "xtp": np.ascontiguousarray(xtp),
                "x2": np.ascontiguousarray(x2),
                "xdiag": np.ascontiguousarray(xdiag),
                "w0c": w0c,
                "w1c": np.ascontiguousarray(w1c),
                "w2c": w2c,
                "ident": ident,
            }
        )
    return in_maps


def _postprocess(results):
    # out_nb [3, 128 n, 256 b] per core -> [B, 384]
    outs = [
        np.asarray(r["out_nb"]).transpose(2, 0, 1).reshape(BL, 3 * FN)
        for r in results
    ]
    return np.ascontiguousarray(np.concatenate(outs, axis=0)).astype(np.float32)


def kernel(x, W0, W1, W2, _trace=False, _trace_kwargs=None):
    if "nc" not in _CACHE:
        _CACHE["nc"] = _build_program()
    nc = _CACHE["nc"]
    in_maps = _prep_inputs(
        np.asarray(x, dtype=np.float32),
        np.asarray(W0, dtype=np.float32),
        np.asarray(W1, dtype=np.float32),
        np.asarray(W2, dtype=np.float32),
    )
    kw = {}
    if _trace:
        kw["trace"] = True
        kw.update(_trace_kwargs or {})
    res = run_bass_kernel_spmd(nc, in_maps, core_ids=list(range(NCORES)), **kw)
    out = _postprocess(res.results)
    if _trace:
        _CACHE["last_results"] = res
    return out


# revision 11
# speedup vs baseline: 1.3019x; 1.0035x over previous
"""CIN (Compressed Interaction Network) Trainium2 kernel.

Reference computation (per batch row b, emb dim d):
    h0 = x                                  [B, 64, 16]
    h_l[b,n,d] = sum_{i,j} x[b,i,d] * h_{l-1}[b,j,d] * Wl[i*Fi+j, n]
    out = concat([sum_d h1, sum_d h2, sum_d h3], axis=1)   [B, 384]

Strategy (pure data parallel over 8 cores, B_loc = 256):
  * Everything lives in "field-major" layout [field, (b,d)] with
    c = b*16+d as the free/column axis (C = 4096 per core).
  * A CIN layer is z[n, c] = sum_(ij) W[(ij), n] * P[(ij), c] where
    P = Khatri-Rao product P[(i,j), c] = X[i,c]*H[j,c].  P row-chunks
    of 128 are built on VectorE as bf16 tensor_tensor multiplies
    against "pair tiles" (X[2t] replicated on partitions 0:64 and
    X[2t+1] on 64:128, pre-materialized on the host and DMAed as
    contiguous 512 KB blocks), then contracted on TensorE with PSUM
    accumulation over the (ij) chunks.
  * Pair-tile sharing: layer 1 uses chunk rows (2t, j<64)+(2t+1, j<64)
    and layer 2 uses half-chunks (2t, j-half)+(2t+1, j-half) against
    partition-duplicated H1 halves, so both layers consume the same
    pair-tile bytes (HBM broadcast traffic 64 MB/core vs 96 naive).
  * Layer 3 only needs the d-summed output, so it is restructured as
    out3[b,:] = vec(G2[b]) @ W2 with G2[b,i,j] = sum_d x[b,i,d]*h2[b,j,d],
    computed with PE transposes of h2 + block-diagonal matmuls against
    a host-prepared block-diagonal x tensor — no layer-3 Khatri-Rao.
    Layer-3 work is interleaved per column block to keep TensorE dense.
  * Host side pre-transposes/casts x and pre-chunks the weights.
"""

import sys

import numpy as np

try:
    import concourse.bass as bass  # noqa: F401
except ImportError:  # grading env fallback
    sys.path.insert(0, "/opt/trn_rl_repo")

import ml_dtypes
import concourse.bacc as bacc
import concourse.bass as bass
import concourse.mybir as mybir
import concourse.tile as tile
from concourse.bass_utils import run_bass_kernel_spmd

BF16 = mybir.dt.bfloat16
F32 = mybir.dt.float32

B, F0, D = 2048, 64, 16
NCORES = 8
BL = B // NCORES          # 256 batch rows per core
C = BL * D                # 4096 columns (b, d)
FN = 128                  # layer width (all three CIN layers)
CT = 512                  # matmul N tile (one PSUM bank of fp32)
CB = 2048                 # column block
NBLK = C // CB            # 2
NCT = CB // CT            # 4 column tiles per block
NPAIR = F0 // 2           # 32 pair tiles
NG = BL // 8              # 32 groups of 8 batch rows (layer-3 path)
NGH = NG // NBLK          # 16 layer-3 groups per block
L1_CHUNKS = (F0 * F0) // 128   # 32
L2_CHUNKS = F0                 # 64 (pair x j-half)

_CACHE = {}


def _build_program():
    nc = bacc.Bacc(None, target_bir_lowering=False)

    xtp_d = nc.dram_tensor("xtp", [NBLK, NPAIR, 128, CB], BF16, kind="ExternalInput")
    x2_d = nc.dram_tensor("x2", [128, C], BF16, kind="ExternalInput")
    xdiag_d = nc.dram_tensor("xdiag", [128, NG * 512], BF16, kind="ExternalInput")
    w0_d = nc.dram_tensor("w0c", [128, L1_CHUNKS * FN], BF16, kind="ExternalInput")
    w1_d = nc.dram_tensor("w1c", [128, L2_CHUNKS * FN], BF16, kind="ExternalInput")
    w2_d = nc.dram_tensor("w2c", [128, F0 * FN], BF16, kind="ExternalInput")
    ident_d = nc.dram_tensor("ident", [128, 128], BF16, kind="ExternalInput")
    out_d = nc.dram_tensor("out_nb", [3, 128, BL], F32, kind="ExternalOutput")

    with tile.TileContext(nc) as tc:
        with (
            tc.tile_pool(name="const", bufs=1) as const,
            tc.tile_pool(name="hbuf", bufs=1) as hbuf,
            tc.tile_pool(name="outs", bufs=1) as outs,
            tc.tile_pool(name="xb", bufs=6) as xbp,
            tc.tile_pool(name="h2x", bufs=2) as h2xp,
            tc.tile_pool(name="pkr", bufs=4) as pkr,
            tc.tile_pool(name="zp", bufs=4, space="PSUM") as zp,
            tc.tile_pool(name="l3sb", bufs=1) as l3sb,
            tc.tile_pool(name="l3ps", bufs=3, space="PSUM") as l3ps,
            tc.tile_pool(name="o3p", bufs=1, space="PSUM") as o3p,
            tc.tile_pool(name="hts", bufs=3) as hts,
            tc.tile_pool(name="xdg", bufs=4) as xdg,
        ):
            # const loads go on ScalarE's DMA queue so they don't delay the
            # pair-tile stream on SyncE's queue at kernel start.
            x2_sb = const.tile([128, C], BF16)
            nc.scalar.dma_start(x2_sb[:], x2_d[:])
            w0_sb = const.tile([128, L1_CHUNKS * FN], BF16)
            nc.scalar.dma_start(w0_sb[:], w0_d[:])
            w1_sb = const.tile([128, L2_CHUNKS * FN], BF16)
            nc.scalar.dma_start(w1_sb[:], w1_d[:])
            w2_sb = const.tile([128, F0 * FN], BF16)
            nc.scalar.dma_start(w2_sb[:], w2_d[:])
            ident_sb = const.tile([128, 128], BF16)
            nc.scalar.dma_start(ident_sb[:], ident_d[:])

            h2_sb = hbuf.tile([128, C], BF16, tag="h2")
            g2t_sb = l3sb.tile([128, NGH * 512], BF16)
            out_sb = outs.tile([128, 3 * BL], F32)

            for blk in range(NBLK):
                c0 = blk * CB
                # ---------------- layer 1 over this block ----------------
                z1 = [
                    zp.tile([128, CT], F32, tag="z", name=f"z1_{blk}_{ct}")
                    for ct in range(NCT)
                ]
                for t in range(NPAIR):
                    xb = xbp.tile([128, CB], BF16, tag="xb", name=f"xb1_{blk}_{t}")
                    nc.sync.dma_start(xb[:], xtp_d[blk, t])
                    p_sb = pkr.tile([128, CB], BF16, tag="p", name=f"p1_{blk}_{t}")
                    nc.vector.tensor_mul(p_sb[:], x2_sb[:, c0 : c0 + CB], xb[:])
                    for ct in range(NCT):
                        nc.tensor.matmul(
                            z1[ct][:],
                            w0_sb[:, t * FN : (t + 1) * FN],
                            p_sb[:, ct * CT : (ct + 1) * CT],
                            start=(t == 0),
                            stop=(t == NPAIR - 1),
                        )

                # z1 copy-out writes straight into the H2x duplication tiles:
                # rows 0:64 of z1 -> h2x_lo[0:64], rows 64:128 -> h2x_hi[0:64];
                # one SBUF->SBUF DMA then fills partitions 64:128 of each.
                h2x_lo = h2xp.tile([128, CB], BF16, tag="h2xlo", name=f"h2xlo_{blk}")
                h2x_hi = h2xp.tile([128, CB], BF16, tag="h2xhi", name=f"h2xhi_{blk}")
                for ct in range(NCT):
                    cs = ct * CT
                    nc.scalar.copy(h2x_lo[0:64, cs : cs + CT], z1[ct][0:64, :])
                    nc.scalar.copy(h2x_hi[0:64, cs : cs + CT], z1[ct][64:128, :])
                    bo = blk * 128 + ct * 32
                    nc.vector.reduce_sum(
                        out_sb[:, bo : bo + 32],
                        z1[ct][:].rearrange("p (b d) -> p b d", d=D),
                        axis=mybir.AxisListType.X,
                    )
                for h2x in (h2x_lo, h2x_hi):
                    nc.gpsimd.dma_start(h2x[64:128, :], h2x[0:64, :])

                # ---------------- layer 2 over this block ----------------
                z2 = [
                    zp.tile([128, CT], F32, tag="z", name=f"z2_{blk}_{ct}")
                    for ct in range(NCT)
                ]
                for t in range(NPAIR):
                    xb = xbp.tile([128, CB], BF16, tag="xb", name=f"xb2_{blk}_{t}")
                    nc.sync.dma_start(xb[:], xtp_d[blk, t])
                    for half, h2x in ((0, h2x_lo), (1, h2x_hi)):
                        k = 2 * t + half
                        p_sb = pkr.tile(
                            [128, CB], BF16, tag="p", name=f"p2_{blk}_{k}"
                        )
                        nc.vector.tensor_mul(p_sb[:], h2x[:], xb[:])
                        for ct in range(NCT):
                            nc.tensor.matmul(
                                z2[ct][:],
                                w1_sb[:, k * FN : (k + 1) * FN],
                                p_sb[:, ct * CT : (ct + 1) * CT],
                                start=(k == 0),
                                stop=(k == L2_CHUNKS - 1),
                            )

                for ct in range(NCT):
                    cc = c0 + ct * CT
                    nc.scalar.copy(h2_sb[:, cc : cc + CT], z2[ct][:])
                    bo = blk * 128 + ct * 32
                    nc.vector.reduce_sum(
                        out_sb[:, BL + bo : BL + bo + 32],
                        z2[ct][:].rearrange("p (b d) -> p b d", d=D),
                        axis=mybir.AxisListType.X,
                    )

                # ------- layer 3 for this block's columns (g groups) ------
                for gl in range(NGH):
                    g = blk * NGH + gl
                    # transpose h2 block: [128 j, 128 (8b,16d)] -> [(8b,16d), j]
                    ht_ps = l3ps.tile([128, 128], BF16, tag="l3", name=f"htps_{g}")
                    nc.tensor.transpose(
                        ht_ps[:], h2_sb[:, g * 128 : (g + 1) * 128], ident_sb[:]
                    )
                    ht_sb = hts.tile([128, 128], BF16, tag="hts", name=f"htsb_{g}")
                    nc.scalar.copy(ht_sb[:], ht_ps[:])

                    xd_sb = xdg.tile([128, 512], BF16, tag="xd", name=f"xd_{g}")
                    nc.scalar.dma_start(xd_sb[:], xdiag_d[:, g * 512 : (g + 1) * 512])

                    # G2T block: out[j, (8b,64i)] = sum_(b',d) h2T x_diag
                    g2_ps = l3ps.tile([128, 512], F32, tag="l3", name=f"g2ps_{g}")
                    nc.tensor.matmul(g2_ps[:], ht_sb[:], xd_sb[:])
                    nc.scalar.copy(g2t_sb[:, gl * 512 : (gl + 1) * 512], g2_ps[:])

                # big contraction for this block's 128 batch rows:
                # out3T[n, (g, bl)] = sum_i W2[i]^T @ G2T_i
                o3_ps = o3p.tile([128, 128], F32, tag="o3", name=f"o3_{blk}")
                g2t_r = g2t_sb[:].rearrange("p (g b i) -> p g b i", b=8, i=F0)
                for i in range(F0):
                    nc.tensor.matmul(
                        o3_ps[:],
                        w2_sb[:, i * FN : (i + 1) * FN],
                        g2t_r[:, :, :, i],
                        start=(i == 0),
                        stop=(i == F0 - 1),
                    )
                nc.scalar.copy(
                    out_sb[:, 2 * BL + blk * 128 : 2 * BL + (blk + 1) * 128],
                    o3_ps[:],
                )

            # ---- store ------------------------------------------------
            for layer in range(3):
                nc.sync.dma_start(
                    out_d[layer], out_sb[:, layer * BL : (layer + 1) * BL]
                )

    nc.finalize()
    return nc


def _prep_inputs(x, W0, W1, W2):
    """Host-side prep: shard x over cores, transpose/cast, chunk weights."""
    bf = ml_dtypes.bfloat16
    xs = np.ascontiguousarray(x).reshape(NCORES, BL, F0, D)

    def chunk_w(W, nchunk):
        # Wc[p, t*FN + n] = W[t*128 + p, n]
        Wc = W.reshape(nchunk, 128, FN).transpose(1, 0, 2).reshape(128, nchunk * FN)
        return np.ascontiguousarray(Wc).astype(bf)

    w0c = chunk_w(W0, L1_CHUNKS)
    w2c = chunk_w(W2, F0)
    # W1 chunk (t, half): p<64 -> W1[2t*128 + half*64 + p],
    #                     p>=64 -> W1[(2t+1)*128 + half*64 + (p-64)]
    W1r = W1.reshape(F0, 2, 64, FN)          # [i, half, j_in_half, n]
    w1c = np.zeros((128, L2_CHUNKS * FN), dtype=bf)
    for t in range(NPAIR):
        for half in range(2):
            k = 2 * t + half
            w1c[0:64, k * FN : (k + 1) * FN] = W1r[2 * t, half].astype(bf)
            w1c[64:128, k * FN : (k + 1) * FN] = W1r[2 * t + 1, half].astype(bf)
    ident = np.eye(128, dtype=np.float32).astype(bf)

    in_maps = []
    for c in range(NCORES):
        xc = xs[c]                                   # [BL, F0, D]
        xt = xc.transpose(1, 0, 2).reshape(F0, C)    # [i, (b d)]
        xt_bf = xt.astype(bf)
        x2 = np.concatenate([xt_bf, xt_bf], axis=0)  # [128, C]

        # pre-replicated pair tiles: xtp[blk, t] = [64 x X[2t]; 64 x X[2t+1]]
        xtb = xt_bf.reshape(F0, NBLK, CB)            # [i, blk, cb]
        xtp = np.repeat(xtb[:, None, :, :], 64, axis=1)   # [i, 64, blk, cb]
        xtp = (
            xtp.reshape(NPAIR, 128, NBLK, CB)
            .transpose(2, 0, 1, 3)                   # [blk, t, 128, cb]
            .copy()
        )

        # xdiag[(bl', d), (g, bl, i)] = x[g*8+bl, i, d] if bl' == bl else 0
        xd = np.zeros((8, D, NG, 8, F0), dtype=bf)
        xg = xc.reshape(NG, 8, F0, D)                # [g, bl, i, d]
        for bl in range(8):
            xd[bl, :, :, bl, :] = xg[:, bl].transpose(2, 0, 1).astype(bf)
        xdiag = xd.reshape(128, NG * 512)

        in_maps.append(
            {
                "xtp": np.ascontiguousarray(xtp),
                "x2": np.ascontiguousarray(x2),
                "xdiag": np.ascontiguousarray(xdiag),
                "w0c": w0c,
                "w1c": np.ascontiguousarray(w1c),
                "w2c": w2c,
                "ident": ident,
            }
        )
    return in_maps


def _postprocess(results):
    # out_nb [3, 128 n, 256 b] per core -> [B, 384]
    outs = [
        np.asarray(r["out_nb"]).transpose(2, 0, 1).reshape(BL, 3 * FN)
        for r in results
    ]
    return np.ascontiguousarray(np.concatenate(outs, axis=0)).astype(np.float32)


def kernel(x, W0, W1, W2, _trace=False, _trace_kwargs=None):
    if "nc" not in _CACHE:
        _CACHE["nc"] = _build_program()
    nc = _CACHE["nc"]
    in_maps = _prep_inputs(
        np.asarray(x, dtype=np.float32),
        np.asarray(W0, dtype=np.float32),
        np.asarray(W1, dtype=np.float32),
        np.asarray(W2, dtype=np.float32),
    )
    kw = {}
    if _trace:
        kw["trace"] = True
        kw.update(_trace_kwargs or {})
    res = run_bass_kernel_spmd(nc, in_maps, core_ids=list(range(NCORES)), **kw)
    out = _postprocess(res.results)
    if _trace:
        _CACHE["last_results"] = res
    return out


# revision 12
# speedup vs baseline: 1.3375x; 1.0274x over previous
"""CIN (Compressed Interaction Network) Trainium2 kernel.

Reference computation (per batch row b, emb dim d):
    h0 = x                                  [B, 64, 16]
    h_l[b,n,d] = sum_{i,j} x[b,i,d] * h_{l-1}[b,j,d] * Wl[i*Fi+j, n]
    out = concat([sum_d h1, sum_d h2, sum_d h3], axis=1)   [B, 384]

Strategy (pure data parallel over 8 cores, B_loc = 256):
  * Everything lives in "field-major" layout [field, (b,d)] with
    c = b*16+d as the free/column axis (C = 4096 per core).
  * A CIN layer is z[n, c] = sum_(ij) W[(ij), n] * P[(ij), c] where
    P = Khatri-Rao product P[(i,j), c] = X[i,c]*H[j,c].  P row-chunks
    of 128 are built on VectorE as bf16 tensor_tensor multiplies
    against "pair tiles" (X[2t] replicated on partitions 0:64 and
    X[2t+1] on 64:128, pre-materialized on the host and DMAed as
    contiguous 256 KB blocks), then contracted on TensorE with PSUM
    accumulation over the (ij) chunks.
  * Pair-tile sharing: columns are processed in four blocks of 1024
    and a block's 32 pair tiles stay resident in SBUF, so layer 1
    (chunk rows (2t, j<64)+(2t+1, j<64)) and layer 2 (half-chunks
    (2t, j-half)+(2t+1, j-half) against partition-duplicated H1
    halves) consume the same broadcast bytes: 32 MB of HBM pair
    traffic per core vs 96 MB naive.
  * Layer 3 only needs the d-summed output, so it is restructured as
    out3[b,:] = vec(G2[b]) @ W2 with G2[b,i,j] = sum_d x[b,i,d]*h2[b,j,d],
    computed with PE transposes of h2 + block-diagonal matmuls against
    a host-prepared block-diagonal x tensor — no layer-3 Khatri-Rao.
    Layer-3 work is interleaved per column block to keep TensorE dense.
  * Host side pre-transposes/casts x and pre-chunks the weights.
"""

import sys

import numpy as np

try:
    import concourse.bass as bass  # noqa: F401
except ImportError:  # grading env fallback
    sys.path.insert(0, "/opt/trn_rl_repo")

import ml_dtypes
import concourse.bacc as bacc
import concourse.bass as bass
import concourse.mybir as mybir
import concourse.tile as tile
from concourse.bass_utils import run_bass_kernel_spmd

BF16 = mybir.dt.bfloat16
F32 = mybir.dt.float32

B, F0, D = 2048, 64, 16
NCORES = 8
BL = B // NCORES          # 256 batch rows per core
C = BL * D                # 4096 columns (b, d)
FN = 128                  # layer width (all three CIN layers)
CT = 512                  # matmul N tile (one PSUM bank of fp32)
CB = 1024                 # column block (pair tiles resident per block)
NBLK = C // CB            # 4
NCT = CB // CT            # 2 column tiles per block
NPAIR = F0 // 2           # 32 pair tiles
NG = BL // 8              # 32 groups of 8 batch rows (layer-3 path)
NGB = CB // 128           # 8 layer-3 groups per block
L1_CHUNKS = (F0 * F0) // 128   # 32
L2_CHUNKS = F0                 # 64 (pair x j-half)

_CACHE = {}


def _build_program():
    nc = bacc.Bacc(None, target_bir_lowering=False)

    xtp_d = nc.dram_tensor("xtp", [NBLK, NPAIR, 128, CB], BF16, kind="ExternalInput")
    x2_d = nc.dram_tensor("x2", [128, C], BF16, kind="ExternalInput")
    xdiag_d = nc.dram_tensor("xdiag", [128, NG * 512], BF16, kind="ExternalInput")
    w0_d = nc.dram_tensor("w0c", [128, L1_CHUNKS * FN], BF16, kind="ExternalInput")
    w1_d = nc.dram_tensor("w1c", [128, L2_CHUNKS * FN], BF16, kind="ExternalInput")
    w2_d = nc.dram_tensor("w2c", [128, F0 * FN], BF16, kind="ExternalInput")
    ident_d = nc.dram_tensor("ident", [128, 128], BF16, kind="ExternalInput")
    out_d = nc.dram_tensor("out_nb", [3, 128, BL], F32, kind="ExternalOutput")

    with tile.TileContext(nc) as tc:
        with (
            tc.tile_pool(name="const", bufs=1) as const,
            tc.tile_pool(name="hbuf", bufs=1) as hbuf,
            tc.tile_pool(name="outs", bufs=1) as outs,
            tc.tile_pool(name="pairs", bufs=NPAIR + 4) as pairs,
            tc.tile_pool(name="h2x", bufs=2) as h2xp,
            tc.tile_pool(name="pkr", bufs=4) as pkr,
            tc.tile_pool(name="zp", bufs=4, space="PSUM") as zp,
            tc.tile_pool(name="l3sb", bufs=2) as l3sb,
            tc.tile_pool(name="l3ps", bufs=3, space="PSUM") as l3ps,
            tc.tile_pool(name="o3p", bufs=1, space="PSUM") as o3p,
            tc.tile_pool(name="hts", bufs=3) as hts,
            tc.tile_pool(name="xdg", bufs=4) as xdg,
        ):
            # const loads go on ScalarE's DMA queue so they don't delay the
            # pair-tile stream on SyncE's queue at kernel start.
            x2_sb = const.tile([128, C], BF16)
            nc.scalar.dma_start(x2_sb[:], x2_d[:])
            w0_sb = const.tile([128, L1_CHUNKS * FN], BF16)
            nc.scalar.dma_start(w0_sb[:], w0_d[:])
            w1_sb = const.tile([128, L2_CHUNKS * FN], BF16)
            nc.scalar.dma_start(w1_sb[:], w1_d[:])
            w2_sb = const.tile([128, F0 * FN], BF16)
            nc.scalar.dma_start(w2_sb[:], w2_d[:])
            ident_sb = const.tile([128, 128], BF16)
            nc.scalar.dma_start(ident_sb[:], ident_d[:])

            h2_sb = hbuf.tile([128, C], BF16, tag="h2")
            out_sb = outs.tile([128, 3 * BL], F32)

            for blk in range(NBLK):
                c0 = blk * CB
                half_idx = blk // 2           # layer-3 half (0 or 1)
                # g2t for a half: 16 groups x 512 cols
                if blk % 2 == 0:
                    g2t_sb = l3sb.tile(
                        [128, 2 * NGB * 512], BF16, tag="g2t", name=f"g2t_{half_idx}"
                    )

                # ---------------- layer 1 over this block ----------------
                z1 = [
                    zp.tile([128, CT], F32, tag="z", name=f"z1_{blk}_{ct}")
                    for ct in range(NCT)
                ]
                pair_tiles = []
                for t in range(NPAIR):
                    xb = pairs.tile([128, CB], BF16, tag="xb", name=f"xb_{blk}_{t}")
                    nc.sync.dma_start(xb[:], xtp_d[blk, t])
                    pair_tiles.append(xb)
                    p_sb = pkr.tile([128, CB], BF16, tag="p", name=f"p1_{blk}_{t}")
                    nc.vector.tensor_mul(p_sb[:], x2_sb[:, c0 : c0 + CB], xb[:])
                    for ct in range(NCT):
                        nc.tensor.matmul(
                            z1[ct][:],
                            w0_sb[:, t * FN : (t + 1) * FN],
                            p_sb[:, ct * CT : (ct + 1) * CT],
                            start=(t == 0),
                            stop=(t == NPAIR - 1),
                        )

                # z1 copy-out writes straight into the H2x duplication tiles:
                # rows 0:64 of z1 -> h2x_lo[0:64], rows 64:128 -> h2x_hi[0:64];
                # one SBUF->SBUF DMA then fills partitions 64:128 of each.
                h2x_lo = h2xp.tile([128, CB], BF16, tag="h2xlo", name=f"h2xlo_{blk}")
                h2x_hi = h2xp.tile([128, CB], BF16, tag="h2xhi", name=f"h2xhi_{blk}")
                for ct in range(NCT):
                    cs = ct * CT
                    nc.scalar.copy(h2x_lo[0:64, cs : cs + CT], z1[ct][0:64, :])
                    nc.scalar.copy(h2x_hi[0:64, cs : cs + CT], z1[ct][64:128, :])
                    bo = blk * (CB // D) + ct * 32
                    nc.vector.reduce_sum(
                        out_sb[:, bo : bo + 32],
                        z1[ct][:].rearrange("p (b d) -> p b d", d=D),
                        axis=mybir.AxisListType.X,
                    )
                for h2x in (h2x_lo, h2x_hi):
                    nc.gpsimd.dma_start(h2x[64:128, :], h2x[0:64, :])

                # ---------------- layer 2 over this block ----------------
                z2 = [
                    zp.tile([128, CT], F32, tag="z", name=f"z2_{blk}_{ct}")
                    for ct in range(NCT)
                ]
                for t in range(NPAIR):
                    for half, h2x in ((0, h2x_lo), (1, h2x_hi)):
                        k = 2 * t + half
                        p_sb = pkr.tile(
                            [128, CB], BF16, tag="p", name=f"p2_{blk}_{k}"
                        )
                        nc.vector.tensor_mul(p_sb[:], h2x[:], pair_tiles[t][:])
                        for ct in range(NCT):
                            nc.tensor.matmul(
                                z2[ct][:],
                                w1_sb[:, k * FN : (k + 1) * FN],
                                p_sb[:, ct * CT : (ct + 1) * CT],
                                start=(k == 0),
                                stop=(k == L2_CHUNKS - 1),
                            )

                for ct in range(NCT):
                    cc = c0 + ct * CT
                    nc.scalar.copy(h2_sb[:, cc : cc + CT], z2[ct][:])
                    bo = blk * (CB // D) + ct * 32
                    nc.vector.reduce_sum(
                        out_sb[:, BL + bo : BL + bo + 32],
                        z2[ct][:].rearrange("p (b d) -> p b d", d=D),
                        axis=mybir.AxisListType.X,
                    )

                # ------- layer 3 for this block's columns (g groups) ------
                for gl in range(NGB):
                    g = blk * NGB + gl
                    gh = (blk % 2) * NGB + gl     # slot within the half buffer
                    # transpose h2 block: [128 j, 128 (8b,16d)] -> [(8b,16d), j]
                    ht_ps = l3ps.tile([128, 128], BF16, tag="l3", name=f"htps_{g}")
                    nc.tensor.transpose(
                        ht_ps[:], h2_sb[:, g * 128 : (g + 1) * 128], ident_sb[:]
                    )
                    ht_sb = hts.tile([128, 128], BF16, tag="hts", name=f"htsb_{g}")
                    nc.scalar.copy(ht_sb[:], ht_ps[:])

                    xd_sb = xdg.tile([128, 512], BF16, tag="xd", name=f"xd_{g}")
                    nc.scalar.dma_start(
                        xd_sb[:], xdiag_d[:, g * 512 : (g + 1) * 512]
                    )

                    # G2T block: out[j, (8b,64i)] = sum_(b',d) h2T x_diag
                    g2_ps = l3ps.tile([128, 512], F32, tag="l3", name=f"g2ps_{g}")
                    nc.tensor.matmul(g2_ps[:], ht_sb[:], xd_sb[:])
                    nc.scalar.copy(g2t_sb[:, gh * 512 : (gh + 1) * 512], g2_ps[:])

                if blk % 2 == 1:
                    # big contraction for this half's 128 batch rows:
                    # out3T[n, (g, bl)] = sum_i W2[i]^T @ G2T_i
                    o3_ps = o3p.tile([128, 128], F32, tag="o3", name=f"o3_{half_idx}")
                    g2t_r = g2t_sb[:].rearrange("p (g b i) -> p g b i", b=8, i=F0)
                    for i in range(F0):
                        nc.tensor.matmul(
                            o3_ps[:],
                            w2_sb[:, i * FN : (i + 1) * FN],
                            g2t_r[:, :, :, i],
                            start=(i == 0),
                            stop=(i == F0 - 1),
                        )
                    nc.scalar.copy(
                        out_sb[
                            :, 2 * BL + half_idx * 128 : 2 * BL + (half_idx + 1) * 128
                        ],
                        o3_ps[:],
                    )

            # ---- store ------------------------------------------------
            for layer in range(3):
                nc.sync.dma_start(
                    out_d[layer], out_sb[:, layer * BL : (layer + 1) * BL]
                )

    nc.finalize()
    return nc


def _prep_inputs(x, W0, W1, W2):
    """Host-side prep: shard x over cores, transpose/cast, chunk weights."""
    bf = ml_dtypes.bfloat16
    xs = np.ascontiguousarray(x).reshape(NCORES, BL, F0, D)

    def chunk_w(W, nchunk):
        # Wc[p, t*FN + n] = W[t*128 + p, n]
        Wc = W.reshape(nchunk, 128, FN).transpose(1, 0, 2).reshape(128, nchunk * FN)
        return np.ascontiguousarray(Wc).astype(bf)

    w0c = chunk_w(W0, L1_CHUNKS)
    w2c = chunk_w(W2, F0)
    # W1 chunk (t, half): p<64 -> W1[2t*128 + half*64 + p],
    #                     p>=64 -> W1[(2t+1)*128 + half*64 + (p-64)]
    W1r = W1.reshape(F0, 2, 64, FN)          # [i, half, j_in_half, n]
    w1c = np.zeros((128, L2_CHUNKS * FN), dtype=bf)
    for t in range(NPAIR):
        for half in range(2):
            k = 2 * t + half
            w1c[0:64, k * FN : (k + 1) * FN] = W1r[2 * t, half].astype(bf)
            w1c[64:128, k * FN : (k + 1) * FN] = W1r[2 * t + 1, half].astype(bf)
    ident = np.eye(128, dtype=np.float32).astype(bf)

    in_maps = []
    for c in range(NCORES):
        xc = xs[c]                                   # [BL, F0, D]
        xt = xc.transpose(1, 0, 2).reshape(F0, C)    # [i, (b d)]
        xt_bf = xt.astype(bf)
        x2 = np.concatenate([xt_bf, xt_bf], axis=0)  # [128, C]

        # pre-replicated pair tiles: xtp[blk, t] = [64 x X[2t]; 64 x X[2t+1]]
        xtb = xt_bf.reshape(F0, NBLK, CB)            # [i, blk, cb]
        xtp = np.repeat(xtb[:, None, :, :], 64, axis=1)   # [i, 64, blk, cb]
        xtp = (
            xtp.reshape(NPAIR, 128, NBLK, CB)
            .transpose(2, 0, 1, 3)                   # [blk, t, 128, cb]
            .copy()
        )

        # xdiag[(bl', d), (g, bl, i)] = x[g*8+bl, i, d] if bl' == bl else 0
        xd = np.zeros((8, D, NG, 8, F0), dtype=bf)
        xg = xc.reshape(NG, 8, F0, D)                # [g, bl, i, d]
        for bl in range(8):
            xd[bl, :, :, bl, :] = xg[:, bl].transpose(2, 0, 1).astype(bf)
        xdiag = xd.reshape(128, NG * 512)

        in_maps.append(
            {
                "xtp": np.ascontiguousarray(xtp),
                "x2": np.ascontiguousarray(x2),
                "xdiag": np.ascontiguousarray(xdiag),
                "w0c": w0c,
                "w1c": np.ascontiguousarray(w1c),
                "w2c": w2c,
                "ident": ident,
            }
        )
    return in_maps


def _postprocess(results):
    # out_nb [3, 128 n, 256 b] per core -> [B, 384]
    outs = [
        np.asarray(r["out_nb"]).transpose(2, 0, 1).reshape(BL, 3 * FN)
        for r in results
    ]
    return np.ascontiguousarray(np.concatenate(outs, axis=0)).astype(np.float32)


def kernel(x, W0, W1, W2, _trace=False, _trace_kwargs=None):
    if "nc" not in _CACHE:
        _CACHE["nc"] = _build_program()
    nc = _CACHE["nc"]
    in_maps = _prep_inputs(
        np.asarray(x, dtype=np.float32),
        np.asarray(W0, dtype=np.float32),
        np.asarray(W1, dtype=np.float32),
        np.asarray(W2, dtype=np.float32),
    )
    kw = {}
    if _trace:
        kw["trace"] = True
        kw.update(_trace_kwargs or {})
    res = run_bass_kernel_spmd(nc, in_maps, core_ids=list(range(NCORES)), **kw)
    out = _postprocess(res.results)
    if _trace:
        _CACHE["last_results"] = res
    return out


# revision 18
# speedup vs baseline: 1.3502x; 1.0094x over previous
"""CIN (Compressed Interaction Network) Trainium2 kernel.

Reference computation (per batch row b, emb dim d):
    h0 = x                                  [B, 64, 16]
    h_l[b,n,d] = sum_{i,j} x[b,i,d] * h_{l-1}[b,j,d] * Wl[i*Fi+j, n]
    out = concat([sum_d h1, sum_d h2, sum_d h3], axis=1)   [B, 384]

Strategy (pure data parallel over 8 cores, B_loc = 256):
  * Everything lives in "field-major" layout [field, (b,d)] with
    c = b*16+d as the free/column axis (C = 4096 per core).
  * A CIN layer is z[n, c] = sum_(ij) W[(ij), n] * P[(ij), c] where
    P = Khatri-Rao product P[(i,j), c] = X[i,c]*H[j,c].  P row-chunks
    of 128 are built on VectorE as bf16 tensor_tensor multiplies
    against "pair tiles" (X[2t] replicated on partitions 0:64 and
    X[2t+1] on 64:128, pre-materialized on the host and DMAed as
    contiguous 256 KB blocks), then contracted on TensorE with PSUM
    accumulation over the (ij) chunks.
  * Pair-tile sharing: columns are processed in four blocks of 1024
    and a block's 32 pair tiles stay resident in SBUF, so layer 1
    (chunk rows (2t, j<64)+(2t+1, j<64)) and layer 2 (half-chunks
    (2t, j-half)+(2t+1, j-half) against partition-duplicated H1
    halves) consume the same broadcast bytes: 32 MB of HBM pair
    traffic per core vs 96 MB naive.
  * Layer 3 only needs the d-summed output, so it is restructured as
    out3[b,:] = vec(G2[b]) @ W2 with G2[b,i,j] = sum_d x[b,i,d]*h2[b,j,d],
    computed with PE transposes of h2 + block-diagonal matmuls against
    a host-prepared block-diagonal x tensor — no layer-3 Khatri-Rao.
    Layer-3 work is interleaved per column block to keep TensorE dense.
  * Host side pre-transposes/casts x and pre-chunks the weights.
"""

import sys

import numpy as np

try:
    import concourse.bass as bass  # noqa: F401
except ImportError:  # grading env fallback
    sys.path.insert(0, "/opt/trn_rl_repo")

import ml_dtypes
import concourse.bacc as bacc
import concourse.bass as bass
import concourse.mybir as mybir
import concourse.tile as tile
from concourse.bass_utils import run_bass_kernel_spmd

BF16 = mybir.dt.bfloat16
F32 = mybir.dt.float32

B, F0, D = 2048, 64, 16
NCORES = 8
BL = B // NCORES          # 256 batch rows per core
C = BL * D                # 4096 columns (b, d)
FN = 128                  # layer width (all three CIN layers)
CT = 512                  # matmul N tile (one PSUM bank of fp32)
CB = 1024                 # column block (pair tiles resident per block)
NBLK = C // CB            # 4
NCT = CB // CT            # 2 column tiles per block
NPAIR = F0 // 2           # 32 pair tiles
NG = BL // 8              # 32 groups of 8 batch rows (layer-3 path)
NGB = CB // 128           # 8 layer-3 groups per block
L1_CHUNKS = (F0 * F0) // 128   # 32
L2_CHUNKS = F0                 # 64 (pair x j-half)

_CACHE = {}


def _build_program():
    nc = bacc.Bacc(None, target_bir_lowering=False)

    xtp_d = nc.dram_tensor("xtp", [NBLK, NPAIR, 128, CB], BF16, kind="ExternalInput")
    x2_d = nc.dram_tensor("x2", [128, C], BF16, kind="ExternalInput")
    xdiag_d = nc.dram_tensor("xdiag", [128, NG * 512], BF16, kind="ExternalInput")
    w0_d = nc.dram_tensor("w0c", [128, L1_CHUNKS * FN], BF16, kind="ExternalInput")
    w1_d = nc.dram_tensor("w1c", [128, L2_CHUNKS * FN], BF16, kind="ExternalInput")
    w2_d = nc.dram_tensor("w2c", [128, F0 * FN], BF16, kind="ExternalInput")
    ident_d = nc.dram_tensor("ident", [128, 128], BF16, kind="ExternalInput")
    out_d = nc.dram_tensor("out_nb", [3, 128, BL], F32, kind="ExternalOutput")

    with tile.TileContext(nc) as tc:
        with (
            tc.tile_pool(name="const", bufs=1) as const,
            tc.tile_pool(name="hbuf", bufs=1) as hbuf,
            tc.tile_pool(name="outs", bufs=1) as outs,
            tc.tile_pool(name="pairs", bufs=NPAIR + 16) as pairs,
            tc.tile_pool(name="h2x", bufs=2) as h2xp,
            tc.tile_pool(name="pkr", bufs=4) as pkr,
            tc.tile_pool(name="zp", bufs=4, space="PSUM") as zp,
            tc.tile_pool(name="l3sb", bufs=1) as l3sb,
            tc.tile_pool(name="l3ps", bufs=3, space="PSUM") as l3ps,
            tc.tile_pool(name="o3p", bufs=1, space="PSUM") as o3p,
            tc.tile_pool(name="hts", bufs=3) as hts,
            tc.tile_pool(name="xdg", bufs=4) as xdg,
        ):
            # const loads go on ScalarE's DMA queue so they don't delay the
            # pair-tile stream on SyncE's queue at kernel start.
            x2_sb = const.tile([128, C], BF16)
            for q in range(NBLK):
                nc.scalar.dma_start(
                    x2_sb[:, q * CB : (q + 1) * CB], x2_d[:, q * CB : (q + 1) * CB]
                )
            w0_sb = const.tile([128, L1_CHUNKS * FN], BF16)
            nc.scalar.dma_start(w0_sb[:], w0_d[:])
            w1_sb = const.tile([128, L2_CHUNKS * FN], BF16)
            nc.scalar.dma_start(w1_sb[:], w1_d[:])
            w2_sb = const.tile([128, F0 * FN], BF16)
            nc.scalar.dma_start(w2_sb[:], w2_d[:])
            ident_sb = const.tile([128, 128], BF16)
            nc.scalar.dma_start(ident_sb[:], ident_d[:])

            h2_sb = hbuf.tile([128, C], BF16, tag="h2")
            out_sb = outs.tile([128, 3 * BL], F32)

            for blk in range(NBLK):
                c0 = blk * CB
                half_idx = blk // 2           # layer-3 half (0 or 1)
                # g2t for a half: 16 groups x 512 cols
                if blk % 2 == 0:
                    g2t_sb = l3sb.tile(
                        [128, 2 * NGB * 512], BF16, tag="g2t", name=f"g2t_{half_idx}"
                    )

                # ---------------- layer 1 over this block ----------------
                z1 = [
                    zp.tile([128, CT], F32, tag="z", name=f"z1_{blk}_{ct}")
                    for ct in range(NCT)
                ]
                pair_tiles = []
                for t in range(NPAIR):
                    xb = pairs.tile([128, CB], BF16, tag="xb", name=f"xb_{blk}_{t}")
                    # alternate between the SyncE and ScalarE HWDGE queues
                    eng = nc.sync if t % 2 == 0 else nc.scalar
                    eng.dma_start(xb[:], xtp_d[blk, t])
                    pair_tiles.append(xb)
                    p_sb = pkr.tile([128, CB], BF16, tag="p", name=f"p1_{blk}_{t}")
                    nc.vector.tensor_mul(p_sb[:], x2_sb[:, c0 : c0 + CB], xb[:])
                    for ct in range(NCT):
                        nc.tensor.matmul(
                            z1[ct][:],
                            w0_sb[:, t * FN : (t + 1) * FN],
                            p_sb[:, ct * CT : (ct + 1) * CT],
                            start=(t == 0),
                            stop=(t == NPAIR - 1),
                        )

                # z1 copy-out writes straight into the H2x duplication tile
                # (columns 0:CB hold the j<64 half, CB:2CB the j>=64 half);
                # one SBUF->SBUF DMA then fills partitions 64:128.
                h2x = h2xp.tile([128, 2 * CB], BF16, tag="h2x", name=f"h2x_{blk}")
                for ct in range(NCT):
                    cs = ct * CT
                    nc.scalar.copy(h2x[0:64, cs : cs + CT], z1[ct][0:64, :])
                    nc.scalar.copy(h2x[0:64, CB + cs : CB + cs + CT], z1[ct][64:128, :])
                    bo = blk * (CB // D) + ct * 32
                    nc.vector.reduce_sum(
                        out_sb[:, bo : bo + 32],
                        z1[ct][:].rearrange("p (b d) -> p b d", d=D),
                        axis=mybir.AxisListType.X,
                    )
                nc.gpsimd.dma_start(h2x[64:128, :], h2x[0:64, :])

                # ---------------- layer 2 over this block ----------------
                z2 = [
                    zp.tile([128, CT], F32, tag="z", name=f"z2_{blk}_{ct}")
                    for ct in range(NCT)
                ]
                for t in range(NPAIR):
                    # one fused TT per pair tile: multiplies both j-halves'
                    # duplicated H1 against the same xb (read twice via a
                    # stride-0 outer free dim).
                    p_sb = pkr.tile(
                        [128, 2 * CB], BF16, tag="p", name=f"p2_{blk}_{t}"
                    )
                    xb_rep = (
                        pair_tiles[t][:].unsqueeze(1).broadcast_to((128, 2, CB))
                    )
                    nc.vector.tensor_mul(
                        p_sb[:].rearrange("p (h c) -> p h c", h=2), h2x[:].rearrange("p (h c) -> p h c", h=2), xb_rep
                    )
                    for half in range(2):
                        k = 2 * t + half
                        for ct in range(NCT):
                            nc.tensor.matmul(
                                z2[ct][:],
                                w1_sb[:, k * FN : (k + 1) * FN],
                                p_sb[
                                    :,
                                    half * CB + ct * CT : half * CB + (ct + 1) * CT,
                                ],
                                start=(k == 0),
                                stop=(k == L2_CHUNKS - 1),
                            )

                for ct in range(NCT):
                    cc = c0 + ct * CT
                    nc.scalar.copy(h2_sb[:, cc : cc + CT], z2[ct][:])
                    bo = blk * (CB // D) + ct * 32
                    nc.vector.reduce_sum(
                        out_sb[:, BL + bo : BL + bo + 32],
                        z2[ct][:].rearrange("p (b d) -> p b d", d=D),
                        axis=mybir.AxisListType.X,
                    )

                # ------- layer 3 for this block's columns (g groups) ------
                for gl in range(NGB):
                    g = blk * NGB + gl
                    gh = (blk % 2) * NGB + gl     # slot within the half buffer
                    # transpose h2 block: [128 j, 128 (8b,16d)] -> [(8b,16d), j]
                    ht_ps = l3ps.tile([128, 128], BF16, tag="l3", name=f"htps_{g}")
                    nc.tensor.transpose(
                        ht_ps[:], h2_sb[:, g * 128 : (g + 1) * 128], ident_sb[:]
                    )
                    ht_sb = hts.tile([128, 128], BF16, tag="hts", name=f"htsb_{g}")
                    nc.scalar.copy(ht_sb[:], ht_ps[:])

                    xd_sb = xdg.tile([128, 512], BF16, tag="xd", name=f"xd_{g}")
                    nc.scalar.dma_start(
                        xd_sb[:], xdiag_d[:, g * 512 : (g + 1) * 512]
                    )

                    # G2T block: out[j, (8b,64i)] = sum_(b',d) h2T x_diag
                    g2_ps = l3ps.tile([128, 512], F32, tag="l3", name=f"g2ps_{g}")
                    nc.tensor.matmul(g2_ps[:], ht_sb[:], xd_sb[:])
                    nc.scalar.copy(g2t_sb[:, gh * 512 : (gh + 1) * 512], g2_ps[:])

                if blk % 2 == 1:
                    # big contraction for this half's 128 batch rows:
                    # out3T[n, (g, bl)] = sum_i W2[i]^T @ G2T_i
                    o3_ps = o3p.tile([128, 128], F32, tag="o3", name=f"o3_{half_idx}")
                    g2t_r = g2t_sb[:].rearrange("p (g b i) -> p g b i", b=8, i=F0)
                    for i in range(F0):
                        nc.tensor.matmul(
                            o3_ps[:],
                            w2_sb[:, i * FN : (i + 1) * FN],
                            g2t_r[:, :, :, i],
                            start=(i == 0),
                            stop=(i == F0 - 1),
                        )
                    nc.scalar.copy(
                        out_sb[
                            :, 2 * BL + half_idx * 128 : 2 * BL + (half_idx + 1) * 128
                        ],
                        o3_ps[:],
                    )

            # ---- store ------------------------------------------------
            for layer in range(3):
                nc.sync.dma_start(
                    out_d[layer], out_sb[:, layer * BL : (layer + 1) * BL]
                )

    nc.finalize()
    return nc


def _prep_inputs(x, W0, W1, W2):
    """Host-side prep: shard x over cores, transpose/cast, chunk weights."""
    bf = ml_dtypes.bfloat16
    xs = np.ascontiguousarray(x).reshape(NCORES, BL, F0, D)

    def chunk_w(W, nchunk):
        # Wc[p, t*FN + n] = W[t*128 + p, n]
        Wc = W.reshape(nchunk, 128, FN).transpose(1, 0, 2).reshape(128, nchunk * FN)
        return np.ascontiguousarray(Wc).astype(bf)

    w0c = chunk_w(W0, L1_CHUNKS)
    w2c = chunk_w(W2, F0)
    # W1 chunk (t, half): p<64 -> W1[2t*128 + half*64 + p],
    #                     p>=64 -> W1[(2t+1)*128 + half*64 + (p-64)]
    W1r = W1.reshape(F0, 2, 64, FN)          # [i, half, j_in_half, n]
    w1c = np.zeros((128, L2_CHUNKS * FN), dtype=bf)
    for t in range(NPAIR):
        for half in range(2):
            k = 2 * t + half
            w1c[0:64, k * FN : (k + 1) * FN] = W1r[2 * t, half].astype(bf)
            w1c[64:128, k * FN : (k + 1) * FN] = W1r[2 * t + 1, half].astype(bf)
    ident = np.eye(128, dtype=np.float32).astype(bf)

    in_maps = []
    for c in range(NCORES):
        xc = xs[c]                                   # [BL, F0, D]
        xt = xc.transpose(1, 0, 2).reshape(F0, C)    # [i, (b d)]
        xt_bf = xt.astype(bf)
        x2 = np.concatenate([xt_bf, xt_bf], axis=0)  # [128, C]

        # pre-replicated pair tiles: xtp[blk, t] = [64 x X[2t]; 64 x X[2t+1]]
        xtb = xt_bf.reshape(F0, NBLK, CB)            # [i, blk, cb]
        xtp = np.repeat(xtb[:, None, :, :], 64, axis=1)   # [i, 64, blk, cb]
        xtp = (
            xtp.reshape(NPAIR, 128, NBLK, CB)
            .transpose(2, 0, 1, 3)                   # [blk, t, 128, cb]
            .copy()
        )

        # xdiag[(bl', d), (g, bl, i)] = x[g*8+bl, i, d] if bl' == bl else 0
        xd = np.zeros((8, D, NG, 8, F0), dtype=bf)
        xg = xc.reshape(NG, 8, F0, D)                # [g, bl, i, d]
        for bl in range(8):
            xd[bl, :, :, bl, :] = xg[:, bl].transpose(2, 0, 1).astype(bf)
        xdiag = xd.reshape(128, NG * 512)

        in_maps.append(
            {
                "xtp": np.ascontiguousarray(xtp),
                "x2": np.ascontiguousarray(x2),
                "xdiag": np.ascontiguousarray(xdiag),
                "w0c": w0c,
                "w1c": np.ascontiguousarray(w1c),
                "w2c": w2c,
                "ident": ident,
            }
        )
    return in_maps


def _postprocess(results):
    # out_nb [3, 128 n, 256 b] per core -> [B, 384]
    outs = [
        np.asarray(r["out_nb"]).transpose(2, 0, 1).reshape(BL, 3 * FN)
        for r in results
    ]
    return np.ascontiguousarray(np.concatenate(outs, axis=0)).astype(np.float32)


def kernel(x, W0, W1, W2, _trace=False, _trace_kwargs=None):
    if "nc" not in _CACHE:
        _CACHE["nc"] = _build_program()
    nc = _CACHE["nc"]
    in_maps = _prep_inputs(
        np.asarray(x, dtype=np.float32),
        np.asarray(W0, dtype=np.float32),
        np.asarray(W1, dtype=np.float32),
        np.asarray(W2, dtype=np.float32),
    )
    kw = {}
    if _trace:
        kw["trace"] = True
        kw.update(_trace_kwargs or {})
    res = run_bass_kernel_spmd(nc, in_maps, core_ids=list(range(NCORES)), **kw)
    out = _postprocess(res.results)
    if _trace:
        _CACHE["last_results"] = res
    return out
